# revision 8
# baseline (speedup 1.0000x reference)
"""GCN 2-layer message passing on 8 Trainium2 NeuronCores, v2.

Strategy: nodes sharded 8x12500 by dst; per-core dsts are degree-sorted into
128-lane blocks so each dst owns a fixed (lane, block) slot. The host
pre-expands per-edge source features (fp8_e3m4 table rows, device-computed)
into column-major "stripe units" of 512 columns (4 blocks x 128 feats); the
device segment-sum is then a chain of PSUM-accumulating matmuls with a
constant identity as stationary weights -- no on-device gather, no one-hot
builds. Epilogue applies relu(dis[dst] * sum + bias). Two NEFFs (table
build, message passing), each run once per layer; host does all
gather/permute/expand glue between launches.
"""

import os
import numpy as np
import ml_dtypes

import concourse.bass as bass
import concourse.tile as tile
from concourse import bacc, mybir
from concourse.bass_utils import run_bass_kernel_spmd

N = 100000
D = 128
NC = 8
SHARD = 12500
NBLK = 100           # 128-dst blocks per core (12800 padded)
PADN = NBLK * 128    # 12800
NGRP = NBLK // 4     # 25 groups of 4 blocks -> 512-col matmul units
UPC = 32             # stripe units per DMA chunk (32*512 fp8 = 16KB/partition)

_f32 = mybir.dt.float32
_bf16 = mybir.dt.bfloat16
_fp8 = mybir.dt.float8e3       # e3m4: rel err ~1.6%, range +-15.5
_np_fp8 = ml_dtypes.float8_e3m4
_np_bf16 = ml_dtypes.bfloat16

BENCH = bool(int(os.environ.get("KERNEL_BENCH", "0")))
EMULATE = bool(int(os.environ.get("KERNEL_EMULATE", "0")))
EXEC_NS = []

# fp8 range scaling: the table NEFF sees deg/TS^2 so its rsqrt yields TS*dis
# (table rows scaled up into e3m4's normal range); the mp NEFF sees deg*TS^2
# so its rsqrt yields dis/TS, cancelling the factor exactly.
TSCALE = 8.0


def _ap3(t, col0, pbcast=False, fbcast=False, nb=4):
    """3D AP [128, nb, 128] over tile t starting at column col0.

    Default: a plain [128, nb*128] view. pbcast: middle dim stride 0 and the
    tile is [128, >=128] (bias tile broadcast across blocks). fbcast: last dim
    stride 0, tile holds per-(lane, block) scalars (dis broadcast across
    feats).
    """
    ap = t[:].ap
    ps = ap[0][0]
    off = t[:].offset + col0
    if fbcast:
        return bass.AP(tensor=t.tensor, offset=off,
                       ap=[[ps, 128], [1, nb], [0, 128]])
    if pbcast:
        return bass.AP(tensor=t.tensor, offset=off,
                       ap=[[ps, 128], [0, nb], [1, 128]])
    return bass.AP(tensor=t.tensor, offset=off,
                   ap=[[ps, 128], [128, nb], [1, 128]])


def build_tab_nc():
    """tab = (x_shard @ W) * rsqrt(deg) as fp8 rows, [128, NBLK*128] layout
    (lane, block, feat); host reinterprets as [PADN, D] rows."""
    nc = bacc.Bacc("TRN2", target_bir_lowering=False, debug=False,
                   enable_asserts=False, num_devices=NC)
    xT = nc.dram_tensor("xT", [D, PADN], _bf16, kind="ExternalInput").ap()
    W = nc.dram_tensor("W", [D, D], _bf16, kind="ExternalInput").ap()
    degc = nc.dram_tensor("degc", [128, NBLK], _f32, kind="ExternalInput").ap()
    tabo = nc.dram_tensor("tabo", [128, NBLK * D], _fp8,
                          kind="ExternalOutput").ap()

    GPC = 5  # groups per IO chunk
    with tile.TileContext(nc) as tc:
        with tc.tile_pool(name="sing", bufs=1) as sing, \
             tc.tile_pool(name="ps", bufs=4, space="PSUM") as ps:
            xt = sing.tile([128, PADN], _bf16)
            w = sing.tile([128, D], _bf16)
            nc.sync.dma_start(out=w[:], in_=W[:])
            dc = sing.tile([128, NBLK], _f32)
            nc.sync.dma_start(out=dc[:], in_=degc[:])
            for i in range(NGRP // GPC):
                sl = slice(i * GPC * 512, (i + 1) * GPC * 512)
                nc.sync.dma_start(out=xt[:, sl], in_=xT[:, sl])
            dsq = sing.tile([128, NBLK], _f32)
            nc.scalar.activation(dsq[:], dc[:],
                                 mybir.ActivationFunctionType.Sqrt)
            dis = sing.tile([128, NBLK], _f32)
            nc.vector.reciprocal(dis[:], dsq[:])
            tbuf = sing.tile([128, NBLK * D], _fp8)
            for g in range(NGRP):
                p = ps.tile([128, 512], _f32, space="PSUM", tag="p")
                for q in range(4):
                    b = 4 * g + q
                    nc.tensor.matmul(out=p[:, q * 128:(q + 1) * 128],
                                     lhsT=xt[:, b * 128:(b + 1) * 128],
                                     rhs=w[:], start=True, stop=True)
                nc.vector.tensor_tensor(
                    out=_ap3(tbuf, g * 512),
                    in0=_ap3(p, 0),
                    in1=_ap3(dis, 4 * g, fbcast=True),
                    op=mybir.AluOpType.mult)
                if g % GPC == GPC - 1:
                    sl = slice((g - GPC + 1) * 512, (g + 1) * 512)
                    nc.sync.dma_start(out=tabo[:, sl], in_=tbuf[:, sl])
    nc.compile()
    return nc


def build_mp_nc(unit_meta, W_msgs, fuse_tab=False):
    """Message passing: PSUM-accumulate pre-expanded message stripes, then
    relu(dis * sum + bias).

    unit_meta: list of (g, k, S_g) per stripe unit, group-major order.
    fuse_tab: additionally compute the next layer's table
    (h @ W2) * TSCALE*dis on-device (PE transpose of each h block via
    identity matmul, then matmul with W2); output tabo (fp8) replaces
    hout (bf16).
    """
    nc = bacc.Bacc("TRN2", target_bir_lowering=False, debug=False,
                   enable_asserts=False, num_devices=NC)
    msgs = nc.dram_tensor("msgs", [128, W_msgs], _fp8,
                          kind="ExternalInput").ap()
    degc = nc.dram_tensor("degc", [128, NBLK], _f32, kind="ExternalInput").ap()
    biasT = nc.dram_tensor("biasT", [128, D], _f32, kind="ExternalInput").ap()
    ident = nc.dram_tensor("ident", [128, 128], _fp8,
                           kind="ExternalInput").ap()
    if fuse_tab:
        identb = nc.dram_tensor("identb", [128, 128], _bf16,
                                kind="ExternalInput").ap()
        W2 = nc.dram_tensor("W2", [D, D], _bf16, kind="ExternalInput").ap()
        degct = nc.dram_tensor("degct", [128, NBLK], _f32,
                               kind="ExternalInput").ap()
        tabo = nc.dram_tensor("tabo", [128, NBLK * D], _fp8,
                              kind="ExternalOutput").ap()
    else:
        hout = nc.dram_tensor("hout", [128, NBLK * D], _bf16,
                              kind="ExternalOutput").ap()

    U = len(unit_meta)
    with tile.TileContext(nc) as tc:
        with tc.tile_pool(name="sing", bufs=1) as sing, \
             tc.tile_pool(name="mchunk", bufs=5) as mchunk, \
             tc.tile_pool(name="e1", bufs=4) as e1, \
             tc.tile_pool(name="e2", bufs=4) as e2, \
             tc.tile_pool(name="htp", bufs=5) as htp, \
             tc.tile_pool(name="ps", bufs=(4 if fuse_tab else 5),
                          space="PSUM") as ps, \
             tc.tile_pool(name="pst", bufs=2, space="PSUM") as pst, \
             tc.tile_pool(name="ps2", bufs=2, space="PSUM") as ps2:
            idt = sing.tile([128, 128], _fp8)
            nc.sync.dma_start(out=idt[:], in_=ident[:])
            bt = sing.tile([128, D], _f32)
            nc.sync.dma_start(out=bt[:], in_=biasT[:])
            dc = sing.tile([128, NBLK], _f32)
            nc.sync.dma_start(out=dc[:], in_=degc[:])
            dsq = sing.tile([128, NBLK], _f32)
            nc.scalar.activation(dsq[:], dc[:],
                                 mybir.ActivationFunctionType.Sqrt)
            dis = sing.tile([128, NBLK], _f32)
            nc.vector.reciprocal(dis[:], dsq[:])
            hbuf = sing.tile([128, NBLK * D], _bf16)
            if fuse_tab:
                idb = sing.tile([128, 128], _bf16)
                nc.sync.dma_start(out=idb[:], in_=identb[:])
                w2 = sing.tile([128, D], _bf16)
                nc.sync.dma_start(out=w2[:], in_=W2[:])
                dct = sing.tile([128, NBLK], _f32)
                nc.sync.dma_start(out=dct[:], in_=degct[:])
                dsqt = sing.tile([128, NBLK], _f32)
                nc.scalar.activation(dsqt[:], dct[:],
                                     mybir.ActivationFunctionType.Sqrt)
                dist = sing.tile([128, NBLK], _f32)
                nc.vector.reciprocal(dist[:], dsqt[:])
                tbuf = sing.tile([128, NBLK * D], _fp8)

            pend = []

            def fusion_a(g):
                pt = pst.tile([128, 512], _f32, space="PSUM", tag="pt")
                for q in range(4):
                    b = 4 * g + q
                    nc.tensor.matmul(
                        out=pt[:, q * 128:(q + 1) * 128],
                        lhsT=hbuf[:, b * 128:(b + 1) * 128],
                        rhs=idb[:], start=True, stop=True)
                ht = htp.tile([128, 512], _bf16, tag="ht")
                if g % 2 == 0:
                    nc.vector.tensor_copy(ht[:], pt[:])
                else:
                    nc.scalar.activation(
                        ht[:], pt[:], mybir.ActivationFunctionType.Copy)
                return ht

            def fusion_b(g, ht):
                p2 = ps2.tile([128, 512], _f32, space="PSUM", tag="p2")
                for q in range(4):
                    nc.tensor.matmul(
                        out=p2[:, q * 128:(q + 1) * 128],
                        lhsT=ht[:, q * 128:(q + 1) * 128],
                        rhs=w2[:], start=True, stop=True)
                nc.vector.tensor_tensor(
                    out=_ap3(tbuf, g * 512),
                    in0=_ap3(p2, 0),
                    in1=_ap3(dist, 4 * g, fbcast=True),
                    op=mybir.AluOpType.mult)
                if g >= NGRP - 5:
                    sl = slice(g * 512, (g + 1) * 512)
                    nc.sync.dma_start(out=tabo[:, sl], in_=tbuf[:, sl])
                elif g % 5 == 4:
                    sl = slice((g - 4) * 512, (g + 1) * 512)
                    nc.sync.dma_start(out=tabo[:, sl], in_=tbuf[:, sl])

            def emit_fusion(g):
                fusion_b(g, fusion_a(g))

            p = None
            for u0 in range(0, U, UPC):
                used = min(UPC, U - u0)
                mt = mchunk.tile([128, UPC, 512], _fp8, tag="mt")
                nc.sync.dma_start(
                    out=mt[:, :used, :],
                    in_=msgs[:, u0 * 512:(u0 + used) * 512])
                for j in range(used):
                    g, k, Sg = unit_meta[u0 + j]
                    if k == 0:
                        p = ps.tile([128, 512], _f32, space="PSUM", tag="p")
                    nc.tensor.matmul(out=p[:], lhsT=idt[:], rhs=mt[:, j, :],
                                     start=(k == 0), stop=(k == Sg - 1))
                    if k == Sg - 1:
                        t1 = e1.tile([128, 512], _f32, tag="t1")
                        nc.vector.tensor_tensor(
                            out=_ap3(t1, 0), in0=_ap3(p, 0),
                            in1=_ap3(dis, 4 * g, fbcast=True),
                            op=mybir.AluOpType.mult)
                        t2 = e2.tile([128, 512], _f32, tag="t2")
                        eng = nc.gpsimd if fuse_tab else nc.vector
                        eng.tensor_tensor(
                            out=_ap3(t2, 0), in0=_ap3(t1, 0),
                            in1=_ap3(bt, 0, pbcast=True),
                            op=mybir.AluOpType.add)
                        nc.scalar.activation(
                            hbuf[:, g * 512:(g + 1) * 512], t2[:],
                            mybir.ActivationFunctionType.Relu)
                        if fuse_tab:
                            pend.append(g)
                            # emit fusion work 2 groups late so PE never
                            # stalls on the relu chain (PE runs in-order)
                            while pend and pend[0] <= g - 3:
                                emit_fusion(pend.pop(0))
                        elif g >= NGRP - 5:
                            sl = slice(g * 512, (g + 1) * 512)
                            nc.sync.dma_start(out=hout[:, sl],
                                              in_=hbuf[:, sl])
                        elif g % 5 == 4:
                            sl = slice((g - 4) * 512, (g + 1) * 512)
                            nc.sync.dma_start(out=hout[:, sl],
                                              in_=hbuf[:, sl])
            if fuse_tab:
                hts = [(g, fusion_a(g)) for g in pend]
                for g, ht in hts:
                    fusion_b(g, ht)
                pend = []
    nc.compile()
    return nc


def prep(edge_index):
    """Host-side layout. Returns shared layout + per-core index data."""
    src = np.concatenate([edge_index[0], np.arange(N, dtype=np.int64)])
    dst = np.concatenate([edge_index[1], np.arange(N, dtype=np.int64)])
    deg = np.bincount(dst, minlength=N).astype(np.float32)

    cores = []
    S_b_all = np.zeros((NC, NBLK), dtype=np.int64)
    for c in range(NC):
        lo, hi = c * SHARD, (c + 1) * SHARD
        sel = (dst >= lo) & (dst < hi)
        s_ = src[sel].astype(np.int64)
        dloc = (dst[sel] - lo).astype(np.int64)
        degl = deg[lo:hi].astype(np.int64)
        perm = np.argsort(-degl, kind="stable")
        pos = np.empty(SHARD, dtype=np.int64)
        pos[perm] = np.arange(SHARD)
        pe = pos[dloc]                      # position of each edge's dst
        order = np.argsort(pe, kind="stable")
        pe_s = pe[order]
        s_s = s_[order]
        firsts = np.searchsorted(pe_s, pe_s)
        kk = np.arange(len(pe_s)) - firsts  # rank within dst
        # per-block stripe count = degree of first (max-degree) lane
        degp = degl[perm]                   # degrees in position order
        # SHARD=12500 is not a multiple of 128: block b covers positions
        # b*128..b*128+127; blocks >= 97 partially/fully padded.
        sb = np.zeros(NBLK, dtype=np.int64)
        nfull = SHARD // 128                # 97 full blocks
        sb[:nfull] = degp[: nfull * 128 : 128]
        if SHARD % 128:
            sb[nfull] = degp[nfull * 128]   # first lane of partial block
        S_b_all[c] = sb
        cores.append(dict(perm=perm, pe=pe_s, kk=kk, ss=s_s, degp=degp))

    # group stripe counts, shared across cores: groups of 4 blocks; since
    # degrees are sorted desc, group max = first block's count
    S_g = np.zeros(NGRP, dtype=np.int64)
    for g in range(NGRP):
        S_g[g] = max(1, S_b_all[:, 4 * g: 4 * g + 4].max())
    starts = np.zeros(NGRP + 1, dtype=np.int64)
    np.cumsum(S_g, out=starts[1:])
    U = int(starts[-1])

    unit_meta = []
    for g in range(NGRP):
        for k in range(int(S_g[g])):
            unit_meta.append((g, k, int(S_g[g])))

    for c in range(NC):
        cd = cores[c]
        blk = cd["pe"] // 128
        lane = cd["pe"] % 128
        gg = blk // 4
        qq = blk % 4
        uu = starts[gg] + cd["kk"]
        src_idx = np.full((128, U, 4), N, dtype=np.int64)
        src_idx[lane, uu, qq] = cd["ss"]
        cd["src_idx"] = src_idx
        dg = np.ones(PADN, dtype=np.float32)
        dg[:SHARD] = cd["degp"]
        cd["degc"] = dg.reshape(NBLK, 128).T.copy()  # [lane, blk]

    return deg, cores, unit_meta, U, starts, S_g


def _expand_msgs(tab_full_u8, src_idx):
    """tab_full_u8: [N+1, 128] uint8 (fp8 bytes, row N = zeros).
    Returns [128, U*512] uint8."""
    m = tab_full_u8[src_idx]            # [128, U, 4, 128]
    return np.ascontiguousarray(m.reshape(128, -1))


_CACHE = {}


def _emu_tab(xT, W, degc):
    x = xT.astype(np.float32).T                        # [PADN, D]
    h = x @ W.astype(np.float32)
    dis = 1.0 / np.sqrt(degc.T.reshape(-1).astype(np.float32))  # [PADN]
    t = (h * dis[:, None]).astype(_np_fp8)
    # repack to [128, NBLK*D] (lane, blk, feat)
    return np.ascontiguousarray(
        t.view(np.uint8).reshape(NBLK, 128, D).transpose(1, 0, 2)
        .reshape(128, NBLK * D))


def _emu_mp(msgs_u8, degc, biasT, unit_meta, starts, S_g):
    m = msgs_u8.view(_np_fp8).astype(np.float32).reshape(128, -1, 4, 128)
    dis = 1.0 / np.sqrt(degc.astype(np.float32))       # [lane, blk]
    out = np.zeros((128, NBLK, D), dtype=np.float32)
    for g in range(NGRP):
        P = m[:, starts[g]:starts[g + 1]].sum(axis=1)  # [128, 4, 128]
        for q in range(4):
            b = 4 * g + q
            out[:, b] = np.maximum(
                P[:, q] * dis[:, b:b + 1] + biasT[0][None, :], 0.0)
    return np.ascontiguousarray(
        out.reshape(128, NBLK * D).astype(_np_bf16))


def kernel(x, edge_index, W1, b1, W2, b2):
    x = np.asarray(x, dtype=np.float32)
    edge_index = np.asarray(edge_index).astype(np.int64)
    W1 = np.asarray(W1, dtype=np.float32)
    b1 = np.asarray(b1, dtype=np.float32)
    W2 = np.asarray(W2, dtype=np.float32)
    b2 = np.asarray(b2, dtype=np.float32)

    deg, cores, unit_meta, U, starts, S_g = prep(edge_index)
    W_msgs = U * 512

    if not EMULATE:
        if "tab" not in _CACHE:
            _CACHE["tab"] = build_tab_nc()
        if "mptab" not in _CACHE:
            _CACHE["mptab"] = build_mp_nc(unit_meta, W_msgs, fuse_tab=True)
        if "mp" not in _CACHE:
            _CACHE["mp"] = build_mp_nc(unit_meta, W_msgs)
        tab_nc, mptab_nc, mp_nc = (_CACHE["tab"], _CACHE["mptab"],
                                   _CACHE["mp"])
    core_ids = list(range(NC))
    ident = np.ascontiguousarray(np.eye(128, dtype=np.float32)
                                 .astype(_np_fp8))

    def run_tab(xTs, W):
        Wb = np.ascontiguousarray(W.astype(_np_bf16))
        dsc = [np.ascontiguousarray(cores[c]["degc"] / (TSCALE * TSCALE))
               for c in core_ids]
        if EMULATE:
            return [_emu_tab(xTs[c], Wb, dsc[c]) for c in core_ids]
        in_maps = [{"xT": xTs[c], "W": Wb, "degc": dsc[c]}
                   for c in core_ids]
        print("launch: tab", flush=True)
        res = run_bass_kernel_spmd(tab_nc, in_maps, core_ids, trace=BENCH)
        if BENCH:
            EXEC_NS.append(res.exec_time_ns)
        return [np.asarray(res.results[c]["tabo"]) for c in core_ids]

    def run_mptab(msgs_list, b, Wn):
        biasT = np.ascontiguousarray(
            np.tile(b.astype(np.float32)[None, :], (128, 1)))
        Wb = np.ascontiguousarray(Wn.astype(_np_bf16))
        identb = np.ascontiguousarray(np.eye(128, dtype=np.float32)
                                      .astype(_np_bf16))
        dmp = [np.ascontiguousarray(cores[c]["degc"] * (TSCALE * TSCALE))
               for c in core_ids]
        dtb = [np.ascontiguousarray(cores[c]["degc"] / (TSCALE * TSCALE))
               for c in core_ids]
        if EMULATE:
            outs = []
            for c in core_ids:
                h = _emu_mp(msgs_list[c], dmp[c], biasT,
                            unit_meta, starts, S_g)
                rows = (h.view(_np_bf16).reshape(128, NBLK, D)
                        .transpose(1, 0, 2).reshape(PADN, D))
                xT = np.ascontiguousarray(rows.T.astype(_np_bf16))
                outs.append(_emu_tab(xT, Wb, dtb[c]))
            return outs
        in_maps = [{"msgs": msgs_list[c].view(_np_fp8),
                    "degc": dmp[c], "biasT": biasT, "ident": ident,
                    "identb": identb, "W2": Wb, "degct": dtb[c]}
                   for c in core_ids]
        print("launch: mptab", flush=True)
        res = run_bass_kernel_spmd(mptab_nc, in_maps, core_ids, trace=BENCH)
        if BENCH:
            EXEC_NS.append(res.exec_time_ns)
        return [np.asarray(res.results[c]["tabo"]) for c in core_ids]

    def run_mp(msgs_list, b):
        biasT = np.ascontiguousarray(
            np.tile(b.astype(np.float32)[None, :], (128, 1)))
        dsc = [np.ascontiguousarray(cores[c]["degc"] * (TSCALE * TSCALE))
               for c in core_ids]
        if EMULATE:
            return [_emu_mp(msgs_list[c], dsc[c], biasT,
                            unit_meta, starts, S_g) for c in core_ids]
        in_maps = [{"msgs": msgs_list[c].view(_np_fp8),
                    "degc": dsc[c], "biasT": biasT,
                    "ident": ident} for c in core_ids]
        print("launch: mp", flush=True)
        res = run_bass_kernel_spmd(mp_nc, in_maps, core_ids, trace=BENCH)
        if BENCH:
            EXEC_NS.append(res.exec_time_ns)
        return [np.asarray(res.results[c]["hout"]) for c in core_ids]

    def assemble_table(tab_list):
        """[128, NBLK*D] fp8 per core -> [N+1, 128] uint8 global rows."""
        full = np.zeros((N + 1, D), dtype=np.uint8)
        for c in core_ids:
            rows = (np.asarray(tab_list[c]).view(np.uint8)
                    .reshape(128, NBLK, D).transpose(1, 0, 2)
                    .reshape(PADN, D))
            full[c * SHARD + cores[c]["perm"]] = rows[:SHARD]
        return full

    def hout_rows(h):
        """[128, NBLK*D] bf16 -> [PADN, D] position-ordered rows."""
        return (np.asarray(h).view(_np_bf16).reshape(128, NBLK, D)
                .transpose(1, 0, 2).reshape(PADN, D))

    # layer 1
    xTs = []
    for c in core_ids:
        t = np.zeros((D, PADN), dtype=_np_bf16)
        t[:, :SHARD] = x[c * SHARD + cores[c]["perm"]].astype(_np_bf16).T
        xTs.append(np.ascontiguousarray(t))
    tab1 = run_tab(xTs, W1)
    tfull = assemble_table(tab1)
    msgs1 = [_expand_msgs(tfull, cores[c]["src_idx"]) for c in core_ids]

    # layer 1 mp fused with layer-2 table build
    tab2 = run_mptab(msgs1, b1, W2)
    tfull2 = assemble_table(tab2)
    msgs2 = [_expand_msgs(tfull2, cores[c]["src_idx"]) for c in core_ids]
    h2 = run_mp(msgs2, b2)

    out = np.empty((N, D), dtype=np.float32)
    for c in core_ids:
        rows = hout_rows(h2[c])[:SHARD].astype(np.float32)
        out[c * SHARD + cores[c]["perm"]] = rows
    return out


# revision 10
# speedup vs baseline: 1.0735x; 1.0735x over previous
"""GCN 2-layer message passing on 8 Trainium2 NeuronCores, v2.

Strategy: nodes sharded 8x12500 by dst; per-core dsts are degree-sorted into
128-lane blocks so each dst owns a fixed (lane, block) slot. The host
pre-expands per-edge source features (fp8_e3m4 table rows, device-computed)
into column-major "stripe units" of 512 columns (4 blocks x 128 feats); the
device segment-sum is then a chain of PSUM-accumulating matmuls with a
constant identity as stationary weights -- no on-device gather, no one-hot
builds. Epilogue applies relu(dis[dst] * sum + bias). Two NEFFs (table
build, message passing), each run once per layer; host does all
gather/permute/expand glue between launches.
"""

import os
import numpy as np
import ml_dtypes

import concourse.bass as bass
import concourse.tile as tile
from concourse import bacc, mybir
from concourse.bass_utils import run_bass_kernel_spmd

N = 100000
D = 128
NC = 8
SHARD = 12500
NBLK = 100           # 128-dst blocks per core (12800 padded)
PADN = NBLK * 128    # 12800
NGRP = NBLK // 4     # 25 groups of 4 blocks -> 512-col matmul units
UPC = 32             # stripe units per DMA chunk (32*512 fp8 = 16KB/partition)

_f32 = mybir.dt.float32
_bf16 = mybir.dt.bfloat16
_fp8 = mybir.dt.float8e4       # e4m3 (DoubleRow-capable)
_np_fp8 = ml_dtypes.float8_e4m3
_np_bf16 = ml_dtypes.bfloat16

BENCH = bool(int(os.environ.get("KERNEL_BENCH", "0")))
EMULATE = bool(int(os.environ.get("KERNEL_EMULATE", "0")))
EXEC_NS = []

# fp8 range scaling: the table NEFF sees deg/TS^2 so its rsqrt yields TS*dis
# (table rows scaled up into e3m4's normal range); the mp NEFF sees deg*TS^2
# so its rsqrt yields dis/TS, cancelling the factor exactly.
TSCALE = 8.0


def _ap3(t, col0, pbcast=False, fbcast=False, nb=4):
    """3D AP [128, nb, 128] over tile t starting at column col0.

    Default: a plain [128, nb*128] view. pbcast: middle dim stride 0 and the
    tile is [128, >=128] (bias tile broadcast across blocks). fbcast: last dim
    stride 0, tile holds per-(lane, block) scalars (dis broadcast across
    feats).
    """
    ap = t[:].ap
    ps = ap[0][0]
    off = t[:].offset + col0
    if fbcast:
        return bass.AP(tensor=t.tensor, offset=off,
                       ap=[[ps, 128], [1, nb], [0, 128]])
    if pbcast:
        return bass.AP(tensor=t.tensor, offset=off,
                       ap=[[ps, 128], [0, nb], [1, 128]])
    return bass.AP(tensor=t.tensor, offset=off,
                   ap=[[ps, 128], [128, nb], [1, 128]])


def build_tab_nc():
    """tab = (x_shard @ W) * rsqrt(deg) as fp8 rows, [128, NBLK*128] layout
    (lane, block, feat); host reinterprets as [PADN, D] rows."""
    nc = bacc.Bacc("TRN2", target_bir_lowering=False, debug=False,
                   enable_asserts=False, num_devices=NC)
    xT = nc.dram_tensor("xT", [D, PADN], _bf16, kind="ExternalInput").ap()
    W = nc.dram_tensor("W", [D, D], _bf16, kind="ExternalInput").ap()
    degc = nc.dram_tensor("degc", [128, NBLK], _f32, kind="ExternalInput").ap()
    tabo = nc.dram_tensor("tabo", [128, NBLK * D], _fp8,
                          kind="ExternalOutput").ap()

    GPC = 5  # groups per IO chunk
    with tile.TileContext(nc) as tc:
        with tc.tile_pool(name="sing", bufs=1) as sing, \
             tc.tile_pool(name="ps", bufs=4, space="PSUM") as ps:
            xt = sing.tile([128, PADN], _bf16)
            w = sing.tile([128, D], _bf16)
            nc.sync.dma_start(out=w[:], in_=W[:])
            dc = sing.tile([128, NBLK], _f32)
            nc.sync.dma_start(out=dc[:], in_=degc[:])
            for i in range(NGRP // GPC):
                sl = slice(i * GPC * 512, (i + 1) * GPC * 512)
                nc.sync.dma_start(out=xt[:, sl], in_=xT[:, sl])
            dsq = sing.tile([128, NBLK], _f32)
            nc.scalar.activation(dsq[:], dc[:],
                                 mybir.ActivationFunctionType.Sqrt)
            dis = sing.tile([128, NBLK], _f32)
            nc.vector.reciprocal(dis[:], dsq[:])
            tbuf = sing.tile([128, NBLK * D], _fp8)
            for g in range(NGRP):
                p = ps.tile([128, 512], _f32, space="PSUM", tag="p")
                for q in range(4):
                    b = 4 * g + q
                    nc.tensor.matmul(out=p[:, q * 128:(q + 1) * 128],
                                     lhsT=xt[:, b * 128:(b + 1) * 128],
                                     rhs=w[:], start=True, stop=True)
                nc.vector.tensor_tensor(
                    out=_ap3(tbuf, g * 512),
                    in0=_ap3(p, 0),
                    in1=_ap3(dis, 4 * g, fbcast=True),
                    op=mybir.AluOpType.mult)
                if g % GPC == GPC - 1:
                    sl = slice((g - GPC + 1) * 512, (g + 1) * 512)
                    nc.scalar.dma_start(out=tabo[:, sl], in_=tbuf[:, sl])
    nc.compile()
    return nc


def build_mp_nc(unit_meta, W_msgs, fuse_tab=False):
    """Message passing: PSUM-accumulate pre-expanded message stripes, then
    relu(dis * sum + bias).

    unit_meta: list of (g, k, S_g) per stripe unit, group-major order.
    fuse_tab: additionally compute the next layer's table
    (h @ W2) * TSCALE*dis on-device (PE transpose of each h block via
    identity matmul, then matmul with W2); output tabo (fp8) replaces
    hout (bf16).
    """
    nc = bacc.Bacc("TRN2", target_bir_lowering=False, debug=False,
                   enable_asserts=False, num_devices=NC)
    msgs = nc.dram_tensor("msgs", [128, W_msgs], _fp8,
                          kind="ExternalInput").ap()
    degc = nc.dram_tensor("degc", [128, NBLK], _f32, kind="ExternalInput").ap()
    biasT = nc.dram_tensor("biasT", [128, D], _f32, kind="ExternalInput").ap()
    ident = nc.dram_tensor("ident", [128, 256], _fp8,
                           kind="ExternalInput").ap()
    if fuse_tab:
        identb = nc.dram_tensor("identb", [128, 128], _bf16,
                                kind="ExternalInput").ap()
        W2 = nc.dram_tensor("W2", [D, D], _bf16, kind="ExternalInput").ap()
        degct = nc.dram_tensor("degct", [128, NBLK], _f32,
                               kind="ExternalInput").ap()
        tabo = nc.dram_tensor("tabo", [128, NBLK * D], _fp8,
                              kind="ExternalOutput").ap()
    else:
        hout = nc.dram_tensor("hout", [128, NBLK * D], _bf16,
                              kind="ExternalOutput").ap()

    U = len(unit_meta)
    with tile.TileContext(nc) as tc:
        with tc.tile_pool(name="sing", bufs=1) as sing, \
             tc.tile_pool(name="mchunk", bufs=5) as mchunk, \
             tc.tile_pool(name="e1", bufs=4) as e1, \
             tc.tile_pool(name="e2", bufs=4) as e2, \
             tc.tile_pool(name="htp", bufs=5) as htp, \
             tc.tile_pool(name="ps", bufs=(4 if fuse_tab else 5),
                          space="PSUM") as ps, \
             tc.tile_pool(name="pst", bufs=2, space="PSUM") as pst, \
             tc.tile_pool(name="ps2", bufs=2, space="PSUM") as ps2:
            idt = sing.tile([128, 256], _fp8)
            nc.sync.dma_start(out=idt[:], in_=ident[:])
            bt = sing.tile([128, D], _f32)
            nc.sync.dma_start(out=bt[:], in_=biasT[:])
            dc = sing.tile([128, NBLK], _f32)
            nc.sync.dma_start(out=dc[:], in_=degc[:])
            dsq = sing.tile([128, NBLK], _f32)
            nc.scalar.activation(dsq[:], dc[:],
                                 mybir.ActivationFunctionType.Sqrt)
            dis = sing.tile([128, NBLK], _f32)
            nc.vector.reciprocal(dis[:], dsq[:])
            hbuf = sing.tile([128, NBLK * D], _bf16)
            if fuse_tab:
                idb = sing.tile([128, 128], _bf16)
                nc.sync.dma_start(out=idb[:], in_=identb[:])
                w2 = sing.tile([128, D], _bf16)
                nc.sync.dma_start(out=w2[:], in_=W2[:])
                dct = sing.tile([128, NBLK], _f32)
                nc.sync.dma_start(out=dct[:], in_=degct[:])
                dsqt = sing.tile([128, NBLK], _f32)
                nc.scalar.activation(dsqt[:], dct[:],
                                     mybir.ActivationFunctionType.Sqrt)
                dist = sing.tile([128, NBLK], _f32)
                nc.vector.reciprocal(dist[:], dsqt[:])
                tbuf = sing.tile([128, NBLK * D], _fp8)

            pend = []

            def fusion_a(g):
                pt = pst.tile([128, 512], _f32, space="PSUM", tag="pt")
                for q in range(4):
                    b = 4 * g + q
                    nc.tensor.matmul(
                        out=pt[:, q * 128:(q + 1) * 128],
                        lhsT=hbuf[:, b * 128:(b + 1) * 128],
                        rhs=idb[:], start=True, stop=True)
                ht = htp.tile([128, 512], _bf16, tag="ht")
                if g % 2 == 0:
                    nc.vector.tensor_copy(ht[:], pt[:])
                else:
                    nc.scalar.activation(
                        ht[:], pt[:], mybir.ActivationFunctionType.Copy)
                return ht

            def fusion_b(g, ht):
                p2 = ps2.tile([128, 512], _f32, space="PSUM", tag="p2")
                for q in range(4):
                    nc.tensor.matmul(
                        out=p2[:, q * 128:(q + 1) * 128],
                        lhsT=ht[:, q * 128:(q + 1) * 128],
                        rhs=w2[:], start=True, stop=True)
                nc.vector.tensor_tensor(
                    out=_ap3(tbuf, g * 512),
                    in0=_ap3(p2, 0),
                    in1=_ap3(dist, 4 * g, fbcast=True),
                    op=mybir.AluOpType.mult)

            def emit_fusion(g):
                fusion_b(g, fusion_a(g))

            p = None
            for u0 in range(0, U, UPC):
                used = min(UPC, U - u0)
                mt = mchunk.tile([128, UPC, 512], _fp8, tag="mt")
                nc.sync.dma_start(
                    out=mt[:, :used, :],
                    in_=msgs[:, u0 * 512:(u0 + used) * 512])
                for j in range(used):
                    g, k, Sg = unit_meta[u0 + j]
                    if k % 2 == 1:
                        continue    # consumed by its pair's DoubleRow
                    if k == 0:
                        p = ps.tile([128, 512], _f32, space="PSUM", tag="p")
                    mtap = mt[:].ap
                    rhs2 = bass.AP(tensor=mt.tensor,
                                   offset=mt[:].offset + j * 512,
                                   ap=[[mtap[0][0], 128], [512, 2],
                                       [1, 512]])
                    idap = idt[:].ap
                    lhs2 = bass.AP(tensor=idt.tensor, offset=idt[:].offset,
                                   ap=[[idap[0][0], 128], [128, 2],
                                       [1, 128]])
                    nc.tensor.matmul(out=p[:], lhsT=lhs2, rhs=rhs2,
                                     start=(k == 0), stop=(k == Sg - 2),
                                     perf_mode=mybir.MatmulPerfMode.DoubleRow)
                    if k == Sg - 2:
                        t1 = e1.tile([128, 512], _f32, tag="t1")
                        nc.vector.tensor_tensor(
                            out=_ap3(t1, 0), in0=_ap3(p, 0),
                            in1=_ap3(dis, 4 * g, fbcast=True),
                            op=mybir.AluOpType.mult)
                        t2 = e2.tile([128, 512], _f32, tag="t2")
                        eng = nc.gpsimd if fuse_tab else nc.vector
                        eng.tensor_tensor(
                            out=_ap3(t2, 0), in0=_ap3(t1, 0),
                            in1=_ap3(bt, 0, pbcast=True),
                            op=mybir.AluOpType.add)
                        nc.scalar.activation(
                            hbuf[:, g * 512:(g + 1) * 512], t2[:],
                            mybir.ActivationFunctionType.Relu)
                        if fuse_tab:
                            pend.append(g)
                            # emit fusion work 2 groups late so PE never
                            # stalls on the relu chain (PE runs in-order)
                            while pend and pend[0] <= g - 3:
                                emit_fusion(pend.pop(0))
                        elif g >= NGRP - 5:
                            sl = slice(g * 512, (g + 1) * 512)
                            nc.scalar.dma_start(out=hout[:, sl],
                                                in_=hbuf[:, sl])
                        elif g % 5 == 4:
                            sl = slice((g - 4) * 512, (g + 1) * 512)
                            nc.scalar.dma_start(out=hout[:, sl],
                                                in_=hbuf[:, sl])
            if fuse_tab:
                hts = [(g, fusion_a(g)) for g in pend]
                for g, ht in hts:
                    fusion_b(g, ht)
                pend = []
                nc.sync.dma_start(out=tabo[:], in_=tbuf[:])
    nc.compile()
    return nc


def prep(edge_index):
    """Host-side layout. Returns shared layout + per-core index data."""
    src = np.concatenate([edge_index[0], np.arange(N, dtype=np.int64)])
    dst = np.concatenate([edge_index[1], np.arange(N, dtype=np.int64)])
    deg = np.bincount(dst, minlength=N).astype(np.float32)

    cores = []
    S_b_all = np.zeros((NC, NBLK), dtype=np.int64)
    for c in range(NC):
        lo, hi = c * SHARD, (c + 1) * SHARD
        sel = (dst >= lo) & (dst < hi)
        s_ = src[sel].astype(np.int64)
        dloc = (dst[sel] - lo).astype(np.int64)
        degl = deg[lo:hi].astype(np.int64)
        perm = np.argsort(-degl, kind="stable")
        pos = np.empty(SHARD, dtype=np.int64)
        pos[perm] = np.arange(SHARD)
        pe = pos[dloc]                      # position of each edge's dst
        order = np.argsort(pe, kind="stable")
        pe_s = pe[order]
        s_s = s_[order]
        firsts = np.searchsorted(pe_s, pe_s)
        kk = np.arange(len(pe_s)) - firsts  # rank within dst
        # per-block stripe count = degree of first (max-degree) lane
        degp = degl[perm]                   # degrees in position order
        # SHARD=12500 is not a multiple of 128: block b covers positions
        # b*128..b*128+127; blocks >= 97 partially/fully padded.
        sb = np.zeros(NBLK, dtype=np.int64)
        nfull = SHARD // 128                # 97 full blocks
        sb[:nfull] = degp[: nfull * 128 : 128]
        if SHARD % 128:
            sb[nfull] = degp[nfull * 128]   # first lane of partial block
        S_b_all[c] = sb
        cores.append(dict(perm=perm, pe=pe_s, kk=kk, ss=s_s, degp=degp))

    # group stripe counts, shared across cores: groups of 4 blocks; since
    # degrees are sorted desc, group max = first block's count
    S_g = np.zeros(NGRP, dtype=np.int64)
    for g in range(NGRP):
        s = int(max(1, S_b_all[:, 4 * g: 4 * g + 4].max()))
        S_g[g] = s + (s & 1)    # even: units pair up for DoubleRow
    starts = np.zeros(NGRP + 1, dtype=np.int64)
    np.cumsum(S_g, out=starts[1:])
    U = int(starts[-1])

    unit_meta = []
    for g in range(NGRP):
        for k in range(int(S_g[g])):
            unit_meta.append((g, k, int(S_g[g])))

    for c in range(NC):
        cd = cores[c]
        blk = cd["pe"] // 128
        lane = cd["pe"] % 128
        gg = blk // 4
        qq = blk % 4
        uu = starts[gg] + cd["kk"]
        src_idx = np.full((128, U, 4), N, dtype=np.int64)
        src_idx[lane, uu, qq] = cd["ss"]
        cd["src_idx"] = src_idx
        dg = np.ones(PADN, dtype=np.float32)
        dg[:SHARD] = cd["degp"]
        cd["degc"] = dg.reshape(NBLK, 128).T.copy()  # [lane, blk]

    return deg, cores, unit_meta, U, starts, S_g


def _expand_msgs(tab_full_u8, src_idx):
    """tab_full_u8: [N+1, 128] uint8 (fp8 bytes, row N = zeros).
    Returns [128, U*512] uint8."""
    m = tab_full_u8[src_idx]            # [128, U, 4, 128]
    return np.ascontiguousarray(m.reshape(128, -1))


_CACHE = {}


def _emu_tab(xT, W, degc):
    x = xT.astype(np.float32).T                        # [PADN, D]
    h = x @ W.astype(np.float32)
    dis = 1.0 / np.sqrt(degc.T.reshape(-1).astype(np.float32))  # [PADN]
    t = (h * dis[:, None]).astype(_np_fp8)
    # repack to [128, NBLK*D] (lane, blk, feat)
    return np.ascontiguousarray(
        t.view(np.uint8).reshape(NBLK, 128, D).transpose(1, 0, 2)
        .reshape(128, NBLK * D))


def _emu_mp(msgs_u8, degc, biasT, unit_meta, starts, S_g):
    m = msgs_u8.view(_np_fp8).astype(np.float32).reshape(128, -1, 4, 128)
    dis = 1.0 / np.sqrt(degc.astype(np.float32))       # [lane, blk]
    out = np.zeros((128, NBLK, D), dtype=np.float32)
    for g in range(NGRP):
        P = m[:, starts[g]:starts[g + 1]].sum(axis=1)  # [128, 4, 128]
        for q in range(4):
            b = 4 * g + q
            out[:, b] = np.maximum(
                P[:, q] * dis[:, b:b + 1] + biasT[0][None, :], 0.0)
    return np.ascontiguousarray(
        out.reshape(128, NBLK * D).astype(_np_bf16))


def kernel(x, edge_index, W1, b1, W2, b2):
    x = np.asarray(x, dtype=np.float32)
    edge_index = np.asarray(edge_index).astype(np.int64)
    W1 = np.asarray(W1, dtype=np.float32)
    b1 = np.asarray(b1, dtype=np.float32)
    W2 = np.asarray(W2, dtype=np.float32)
    b2 = np.asarray(b2, dtype=np.float32)

    deg, cores, unit_meta, U, starts, S_g = prep(edge_index)
    W_msgs = U * 512

    if not EMULATE:
        if "tab" not in _CACHE:
            _CACHE["tab"] = build_tab_nc()
        if "mptab" not in _CACHE:
            _CACHE["mptab"] = build_mp_nc(unit_meta, W_msgs, fuse_tab=True)
        if "mp" not in _CACHE:
            _CACHE["mp"] = build_mp_nc(unit_meta, W_msgs)
        tab_nc, mptab_nc, mp_nc = (_CACHE["tab"], _CACHE["mptab"],
                                   _CACHE["mp"])
    core_ids = list(range(NC))
    eye = np.eye(128, dtype=np.float32).astype(_np_fp8)
    ident = np.ascontiguousarray(np.concatenate([eye, eye], axis=1))

    def run_tab(xTs, W):
        Wb = np.ascontiguousarray(W.astype(_np_bf16))
        dsc = [np.ascontiguousarray(cores[c]["degc"] / (TSCALE * TSCALE))
               for c in core_ids]
        if EMULATE:
            return [_emu_tab(xTs[c], Wb, dsc[c]) for c in core_ids]
        in_maps = [{"xT": xTs[c], "W": Wb, "degc": dsc[c]}
                   for c in core_ids]
        print("launch: tab", flush=True)
        res = run_bass_kernel_spmd(tab_nc, in_maps, core_ids, trace=BENCH)
        if BENCH:
            EXEC_NS.append(res.exec_time_ns)
        return [np.asarray(res.results[c]["tabo"]) for c in core_ids]

    def run_mptab(msgs_list, b, Wn):
        biasT = np.ascontiguousarray(
            np.tile(b.astype(np.float32)[None, :], (128, 1)))
        Wb = np.ascontiguousarray(Wn.astype(_np_bf16))
        identb = np.ascontiguousarray(np.eye(128, dtype=np.float32)
                                      .astype(_np_bf16))
        dmp = [np.ascontiguousarray(cores[c]["degc"] * (TSCALE * TSCALE))
               for c in core_ids]
        dtb = [np.ascontiguousarray(cores[c]["degc"] / (TSCALE * TSCALE))
               for c in core_ids]
        if EMULATE:
            outs = []
            for c in core_ids:
                h = _emu_mp(msgs_list[c], dmp[c], biasT,
                            unit_meta, starts, S_g)
                rows = (h.view(_np_bf16).reshape(128, NBLK, D)
                        .transpose(1, 0, 2).reshape(PADN, D))
                xT = np.ascontiguousarray(rows.T.astype(_np_bf16))
                outs.append(_emu_tab(xT, Wb, dtb[c]))
            return outs
        in_maps = [{"msgs": msgs_list[c].view(_np_fp8),
                    "degc": dmp[c], "biasT": biasT, "ident": ident,
                    "identb": identb, "W2": Wb, "degct": dtb[c]}
                   for c in core_ids]
        print("launch: mptab", flush=True)
        res = run_bass_kernel_spmd(mptab_nc, in_maps, core_ids, trace=BENCH)
        if BENCH:
            EXEC_NS.append(res.exec_time_ns)
        return [np.asarray(res.results[c]["tabo"]) for c in core_ids]

    def run_mp(msgs_list, b):
        biasT = np.ascontiguousarray(
            np.tile(b.astype(np.float32)[None, :], (128, 1)))
        dsc = [np.ascontiguousarray(cores[c]["degc"] * (TSCALE * TSCALE))
               for c in core_ids]
        if EMULATE:
            return [_emu_mp(msgs_list[c], dsc[c], biasT,
                            unit_meta, starts, S_g) for c in core_ids]
        in_maps = [{"msgs": msgs_list[c].view(_np_fp8),
                    "degc": dsc[c], "biasT": biasT,
                    "ident": ident} for c in core_ids]
        print("launch: mp", flush=True)
        res = run_bass_kernel_spmd(mp_nc, in_maps, core_ids, trace=BENCH)
        if BENCH:
            EXEC_NS.append(res.exec_time_ns)
        return [np.asarray(res.results[c]["hout"]) for c in core_ids]

    def assemble_table(tab_list):
        """[128, NBLK*D] fp8 per core -> [N+1, 128] uint8 global rows."""
        full = np.zeros((N + 1, D), dtype=np.uint8)
        for c in core_ids:
            rows = (np.asarray(tab_list[c]).view(np.uint8)
                    .reshape(128, NBLK, D).transpose(1, 0, 2)
                    .reshape(PADN, D))
            full[c * SHARD + cores[c]["perm"]] = rows[:SHARD]
        return full

    def hout_rows(h):
        """[128, NBLK*D] bf16 -> [PADN, D] position-ordered rows."""
        return (np.asarray(h).view(_np_bf16).reshape(128, NBLK, D)
                .transpose(1, 0, 2).reshape(PADN, D))

    # layer 1
    xTs = []
    for c in core_ids:
        t = np.zeros((D, PADN), dtype=_np_bf16)
        t[:, :SHARD] = x[c * SHARD + cores[c]["perm"]].astype(_np_bf16).T
        xTs.append(np.ascontiguousarray(t))
    tab1 = run_tab(xTs, W1)
    tfull = assemble_table(tab1)
    msgs1 = [_expand_msgs(tfull, cores[c]["src_idx"]) for c in core_ids]

    # layer 1 mp fused with layer-2 table build
    tab2 = run_mptab(msgs1, b1, W2)
    tfull2 = assemble_table(tab2)
    msgs2 = [_expand_msgs(tfull2, cores[c]["src_idx"]) for c in core_ids]
    h2 = run_mp(msgs2, b2)

    out = np.empty((N, D), dtype=np.float32)
    for c in core_ids:
        rows = hout_rows(h2[c])[:SHARD].astype(np.float32)
        out[c * SHARD + cores[c]["perm"]] = rows
    return out
